# revision 23
# baseline (speedup 1.0000x reference)
"""Trainium2 Bass kernel for the Atari actor-critic agent (CNN -> GRU -> heads).

Sharding: data-parallel over the B=32 env axis -> 4 envs per core x 8 cores.
Per core: CNN on 512 frames, GRU scan over T=128 steps (4 envs), heads.
No cross-core communication.

CNN mapping (no tile_position; all matmuls full-array K=128 where possible):
 - conv1: host im2col; two images pair-stacked on K with block-diagonal
   weights -> K=128, M=[2img x 2rep x 32och].
 - conv2: act1 stored 2D-parity-split [(rowpar, colpar, ch) = 128 partitions]
   (built by 4 strip DMAs from conv1 staging); 4 rounds of K=128 accumulate.
 - conv3: act2 stored as two row-shifted strips [(shift, ch) = 128]
   (built directly by conv2's two evacuation ops); 6 rounds of K=128.
 - fc: 49 accumulating K=64 matmuls per 128-unit tile.
 - gx GEMM fp32r; GRU scan exact fp32; heads fp32.
"""
import numpy as np
import ml_dtypes

import concourse.bass as bass
from concourse import bacc
import concourse.mybir as mybir
from concourse.tile import TileContext
from concourse.bass_utils import run_bass_kernel_spmd

F32 = mybir.dt.float32
F32R = mybir.dt.float32r
BF16 = mybir.dt.bfloat16
AX = mybir.AluOpType
AF = mybir.ActivationFunctionType

T, B, BL, NCORES, H = 128, 32, 4, 8, 128
NIMG = T * BL          # 512 frames per core
NT = 16                # image tiles
NTI = NIMG // NT       # 32 frames per tile
A_DIM = 6

_cache = {}


def build_kernel():
    import os
    skip_cnn = bool(os.environ.get("SKIP_CNN"))
    skip_scan = bool(os.environ.get("SKIP_SCAN"))
    if "nc" in _cache:
        return _cache["nc"]
    nc = bacc.Bacc(None, target_bir_lowering=False, debug=False)

    # ---- DRAM inputs ----
    # conv1 im2col, image pairs stacked on K: [nt, (2img x 64patch), pair, 400]
    xcol = nc.dram_tensor("xcol", (NT, 128, NTI // 2, 400), BF16, kind="ExternalInput")
    w1p = nc.dram_tensor("w1p", (128, 128), BF16, kind="ExternalInput")
    b1r = nc.dram_tensor("b1r", (128, 1), F32, kind="ExternalInput")
    w2l = nc.dram_tensor("w2l", (128, 4, 128), BF16, kind="ExternalInput")
    b2r = nc.dram_tensor("b2r", (128, 1), F32, kind="ExternalInput")
    w3l = nc.dram_tensor("w3l", (128, 6, 64), BF16, kind="ExternalInput")
    b3t = nc.dram_tensor("b3t", (64, 1), F32, kind="ExternalInput")
    fcwl = nc.dram_tensor("fcwl", (4, 64, 49, 128), BF16, kind="ExternalInput")
    fcbt = nc.dram_tensor("fcbt", (128, 4), F32, kind="ExternalInput")
    wihl = nc.dram_tensor("wihl", (128, 4, 384), F32R, kind="ExternalInput")
    bihl = nc.dram_tensor("bihl", (128, 3), F32, kind="ExternalInput")
    whhl = nc.dram_tensor("whhl", (128, 384), F32, kind="ExternalInput")
    awl = nc.dram_tensor("awl", (128, 8), F32, kind="ExternalInput")
    abl = nc.dram_tensor("abl", (8, 1), F32, kind="ExternalInput")
    dmask = nc.dram_tensor("dmask", (128, T * BL), F32, kind="ExternalInput")
    h0t = nc.dram_tensor("h0t", (128, BL), F32, kind="ExternalInput")
    identw = nc.dram_tensor("identw", (128, 128), F32, kind="ExternalInput")

    # ---- DRAM outputs ----
    y_av = nc.dram_tensor("y_av", (8, NIMG), F32, kind="ExternalOutput")
    y_h = nc.dram_tensor("y_h", (128, BL), F32, kind="ExternalOutput")

    with TileContext(nc) as tc:
        with tc.tile_pool(name="wpool", bufs=1) as wp, \
             tc.tile_pool(name="big", bufs=1) as bigp:
            w1_s = wp.tile([128, 128], BF16); nc.sync.dma_start(w1_s[:], w1p[:])
            b1_s = wp.tile([128, 1], F32); nc.sync.dma_start(b1_s[:], b1r[:])
            w2_s = wp.tile([128, 4, 128], BF16); nc.sync.dma_start(w2_s[:], w2l[:])
            b2_s = wp.tile([128, 1], F32); nc.sync.dma_start(b2_s[:], b2r[:])
            w3_s = wp.tile([128, 6, 64], BF16); nc.sync.dma_start(w3_s[:], w3l[:])
            b3_s = wp.tile([64, 1], F32); nc.sync.dma_start(b3_s[:], b3t[:])
            fcb_s = wp.tile([128, 4], F32); nc.sync.dma_start(fcb_s[:], fcbt[:])
            fcw_s = wp.tile([64, 4, 49, 128], BF16)
            nc.sync.dma_start(fcw_s[:], fcwl.rearrange("m p a b -> p m a b"))
            wih_s = wp.tile([128, 4, 384], F32R); nc.sync.dma_start(wih_s[:], wihl[:])
            bih_s = wp.tile([128, 3], F32); nc.sync.dma_start(bih_s[:], bihl[:])
            whh_s = wp.tile([128, 384], F32); nc.sync.dma_start(whh_s[:], whhl[:])
            aw_s = wp.tile([128, 8], F32); nc.sync.dma_start(aw_s[:], awl[:])
            ab_s = wp.tile([8, 1], F32); nc.sync.dma_start(ab_s[:], abl[:])
            dm_s = bigp.tile([128, T * BL], F32); nc.sync.dma_start(dm_s[:], dmask[:])
            h0_s = wp.tile([128, BL], F32); nc.sync.dma_start(h0_s[:], h0t[:])
            id_s = wp.tile([128, 128], F32); nc.sync.dma_start(id_s[:], identw[:])

            act3_s = bigp.tile([64, 2, 64, 49], BF16)  # rolling 2-chunk
            feat_s = bigp.tile([128, 4, NIMG], F32R)
            gx_s = bigp.tile([128, T, 3, BL], F32)
            hs_s = bigp.tile([128, T * BL], F32)
            ns_s = bigp.tile([128, T * BL], F32)
            hm_s = bigp.tile([128, T * BL], F32)
            rz_s = bigp.tile([128, T, 8], F32)

            # ---------------- CNN emission as a generator ----------------
            # yields between op groups so scan steps can interleave
            CH = 2              # nt tiles per fc chunk (64 images, 16 steps)
            NCHUNK = NT // CH   # 8 chunks

            def emit_cnn_chunk(ck, pools):
                xinp, st1p, as1p, as2p, ps1p, ps2p, ps3p, psfc, psgx = pools
                for nt in range(CH * ck, CH * (ck + 1)):
                    xin = xinp.tile([128, NTI // 2, 400], BF16)
                    nc.sync.dma_start(xin[:], xcol[nt])
                    st1 = st1p.tile([128, NTI // 2, 400], BF16)
                    for pr in range(NTI // 2):
                        ps = ps1p.tile([128, 400], F32)
                        nc.tensor.matmul(ps[:], w1_s[:], xin[:, pr, :],
                                         start=True, stop=True)
                        if pr % 2 == 0:
                            nc.scalar.activation(st1[:, pr, :], ps[:], AF.Relu,
                                                 bias=b1_s[:, 0:1])
                        else:
                            nc.vector.tensor_scalar(st1[:, pr, :], ps[:],
                                                    b1_s[:, 0:1], 0.0, AX.add, AX.max)
                        if pr % 4 == 3:
                            yield
                    as1 = as1p.tile([128, NTI, 10, 10], BF16)
                    st1v = st1.rearrange("p a (u v y x) -> p a u v (y x)", u=2, v=2, y=10, x=10)
                    a1v = as1.rearrange("p (a h) y j -> p a h (y j)", h=2)
                    for rp in range(2):
                        for cp in range(2):
                            sp = 2 * rp + cp
                            for half in range(2):
                                p0 = 64 * half + 32 * (sp % 2)
                                nc.sync.dma_start(a1v[32 * sp:32 * sp + 32, :, half, :],
                                                  st1v[p0:p0 + 32, :, rp, cp, :])
                    yield
                    as2 = as2p.tile([128, NTI, 9, 9], BF16)
                    nc.vector.memset(as2[64:128, :, 8, :], 0.0)
                    for cc in range((NTI + 5) // 6):
                        i0 = 6 * cc
                        isz = min(6, NTI - i0)
                        ncols = isz * 81
                        ps2 = ps2p.tile([128, 512], F32)
                        for r in range(4):
                            a, bb = divmod(r, 2)
                            rhs = as1[:, i0:i0 + isz, a:a + 9, bb:bb + 9]
                            nc.tensor.matmul(ps2[:, :ncols], w2_s[:, r, :], rhs,
                                             start=(r == 0), stop=(r == 3))
                        p2v = ps2[:, :ncols].rearrange("p (a y x) -> p a y x", a=isz, y=9)
                        if cc % 2 == 0:
                            nc.scalar.activation(as2[0:64, i0:i0 + isz], p2v[0:64], AF.Relu,
                                                 bias=b2_s[0:64, 0:1])
                            nc.vector.tensor_scalar(as2[64:128, i0:i0 + isz, 0:8, :],
                                                    p2v[64:128, :, 1:9, :],
                                                    b2_s[64:128, 0:1], 0.0, AX.add, AX.max)
                        else:
                            nc.vector.tensor_scalar(as2[0:64, i0:i0 + isz], p2v[0:64],
                                                    b2_s[0:64, 0:1], 0.0, AX.add, AX.max)
                            nc.scalar.activation(as2[64:128, i0:i0 + isz, 0:8, :],
                                                 p2v[64:128, :, 1:9, :], AF.Relu,
                                                 bias=b2_s[64:128, 0:1])
                        yield
                    for cc in range((NTI + 9) // 10):
                        i0 = 10 * cc
                        isz = min(10, NTI - i0)
                        ncols = isz * 49
                        ps3 = ps3p.tile([128, 512], F32)
                        for r in range(6):
                            a, kj = divmod(r, 3)
                            rhs = as2[:, i0:i0 + isz, 2 * a:2 * a + 7, kj:kj + 7]
                            nc.tensor.matmul(ps3[0:64, :ncols], w3_s[:, r, :], rhs,
                                             start=(r == 0), stop=(r == 5))
                        ck_slot = (nt // CH) % 2
                        io = (nt % CH) * NTI + i0
                        dst = act3_s[:, ck_slot, io:io + isz, :]
                        dst = dst.rearrange("p a b -> p (a b)")
                        if cc % 2 == 0:
                            nc.scalar.activation(dst, ps3[0:64, :ncols], AF.Relu,
                                                 bias=b3_s[:, 0:1])
                        else:
                            nc.vector.tensor_scalar(dst, ps3[0:64, :ncols],
                                                    b3_s[:, 0:1], 0.0, AX.add, AX.max)
                        yield
                # fc + gx for this chunk (images [NI0, NI1))
                NI0 = CH * NTI * ck
                NW = CH * NTI
                for m in range(4):
                    ps = psfc.tile([128, NW], F32)
                    for pos in range(49):
                        nc.tensor.matmul(ps[:], fcw_s[:, m, pos, :],
                                         act3_s[:, ck % 2, :, pos],
                                         start=(pos == 0), stop=(pos == 48))
                    if m % 2 == 0:
                        nc.scalar.activation(feat_s[:, m, NI0:NI0 + NW], ps[:], AF.Relu,
                                             bias=fcb_s[:, m:m + 1])
                    else:
                        nc.vector.tensor_scalar(feat_s[:, m, NI0:NI0 + NW], ps[:],
                                                fcb_s[:, m:m + 1], 0.0, AX.add, AX.max)
                    yield
                for g in range(3):
                    ps = psgx.tile([128, NW], F32)
                    for kt in range(4):
                        nc.tensor.matmul(ps[:], wih_s[:, kt, 128 * g:128 * (g + 1)],
                                         feat_s[:, kt, NI0:NI0 + NW],
                                         start=(kt == 0), stop=(kt == 3))
                    nc.vector.tensor_scalar(
                        gx_s[:, 16 * CH * ck // CH * 0 + NI0 // BL:NI0 // BL + NW // BL, g, :],
                        ps[:].rearrange("p (t e) -> p t e", e=BL),
                        bih_s[:, g:g + 1], 0.0, AX.add, AX.bypass)
                    yield
                while True:
                    yield

            # ---------------- GRU scan, CNN interleaved ----------------
            with tc.tile_pool(name="xin", bufs=2) as xinp, \
                 tc.tile_pool(name="st1", bufs=2) as st1p, \
                 tc.tile_pool(name="as1", bufs=2) as as1p, \
                 tc.tile_pool(name="as2", bufs=2) as as2p, \
                 tc.tile_pool(name="ps1", bufs=2, space="PSUM") as ps1p, \
                 tc.tile_pool(name="ps2", bufs=1, space="PSUM") as ps2p, \
                 tc.tile_pool(name="ps3", bufs=1, space="PSUM") as ps3p, \
                 tc.tile_pool(name="psfc", bufs=1, space="PSUM") as psfc, \
                 tc.tile_pool(name="psgx", bufs=1, space="PSUM") as psgx, \
                 tc.tile_pool(name="pssc", bufs=2, space="PSUM") as pssc, \
                 tc.tile_pool(name="scw", bufs=6) as scw, \
                 tc.tile_pool(name="bp", bufs=1) as bpp:
                pools = (xinp, st1p, as1p, as2p, ps1p, ps2p, ps3p, psfc, psgx)
                gens = [emit_cnn_chunk(ck, pools) for ck in range(NCHUNK)]
                if skip_cnn:
                    nc.vector.memset(gx_s[:, :, :, :], 0.0)
                else:
                    for _ in range(200):
                        next(gens[0])
                SPC = T // NCHUNK  # scan steps per chunk
                # hm(0) = h0 * dmask(0)
                nc.vector.tensor_tensor(hm_s[:, 0:BL], h0_s[:], dm_s[:, 0:BL], AX.mult)
                for t in range(T if not skip_scan else 1):
                    hm = hm_s[:, BL * t:BL * (t + 1)]
                    ps = pssc.tile([128, 12], F32)
                    for g in range(3):
                        nc.tensor.matmul(ps[:, 4 * g:4 * (g + 1)],
                                         whh_s[:, 128 * g:128 * (g + 1)], hm,
                                         start=True, stop=True)
                    srz = scw.tile([128, 8], F32, tag="srz")
                    nc.vector.tensor_tensor(
                        srz[:], ps[:, 0:8],
                        gx_s[:, t, 0:2, :].rearrange("p a b -> p (a b)"), AX.add)
                    rz = rz_s[:, t, :]
                    nc.scalar.activation(rz, srz[:], AF.Sigmoid)
                    t1 = scw.tile([128, BL], F32, tag="t1")
                    nc.vector.tensor_tensor(t1[:], ps[:, 8:12], rz_s[:, t, 0:4], AX.mult)
                    t2 = scw.tile([128, BL], F32, tag="t2")
                    nc.vector.tensor_tensor(t2[:], t1[:], gx_s[:, t, 2, :], AX.add)
                    nsb = ns_s[:, BL * t:BL * (t + 1)]
                    nc.scalar.activation(nsb, t2[:], AF.Tanh)
                    if t + 1 < T:
                        # hm(t+1) = dm1*(n + z*(hm - n)) = n*q + p,
                        # q = (1-z)*dm1, p = z*dm1*hm   (q, p off critical path)
                        dm1 = dm_s[:, BL * (t + 1):BL * (t + 2)]
                        za = scw.tile([128, BL], F32, tag="za")
                        nc.vector.tensor_tensor(za[:], rz_s[:, t, 4:8], dm1, AX.mult)
                        qd = scw.tile([128, BL], F32, tag="qd")
                        nc.vector.tensor_tensor(qd[:], dm1, za[:], AX.subtract)
                        pd = scw.tile([128, BL], F32, tag="pd")
                        nc.vector.tensor_tensor(pd[:], za[:], hm, AX.mult)
                        u = scw.tile([128, BL], F32, tag="u")
                        nc.vector.tensor_tensor(u[:], nsb, qd[:], AX.mult)
                        nc.vector.tensor_tensor(hm_s[:, BL * (t + 1):BL * (t + 2)],
                                                u[:], pd[:], AX.add)
                    if not skip_cnn:
                        ck_next = t // SPC + 1
                        if ck_next < NCHUNK:
                            g = gens[ck_next]
                            for _ in range(3):
                                try:
                                    next(g)
                                except StopIteration:
                                    break
                # batched hs = ns + z*(hm - ns)
                for seg in range(2):
                    sl = slice(seg * T * BL // 2, (seg + 1) * T * BL // 2)
                    tsl = slice(seg * T // 2, (seg + 1) * T // 2)
                    bv = bpp.tile([128, T * BL // 2], F32, tag="bv")
                    nc.vector.tensor_tensor(bv[:], hm_s[:, sl], ns_s[:, sl], AX.subtract)
                    bw = bpp.tile([128, T * BL // 2], F32, tag="bw")
                    nc.vector.tensor_tensor(
                        bw[:], bv[:].rearrange("p (t e) -> p t e", e=BL),
                        rz_s[:, tsl, 4:8], AX.mult)
                    nc.vector.tensor_tensor(hs_s[:, sl], bw[:], ns_s[:, sl], AX.add)

            # ---------------- heads ----------------
            with tc.tile_pool(name="psh", bufs=1, space="PSUM") as pshp, \
                 tc.tile_pool(name="ho", bufs=1) as hop:
                psh = pshp.tile([8, NIMG], F32)
                nc.tensor.matmul(psh[:], aw_s[:], hs_s[:], start=True, stop=True)
                osb = hop.tile([8, NIMG], F32)
                nc.vector.tensor_scalar(osb[:], psh[:], ab_s[:, 0:1], 0.0,
                                        AX.add, AX.bypass)
                nc.sync.dma_start(y_av[:], osb[:])
                nc.sync.dma_start(y_h[:], hs_s[:, BL * (T - 1):BL * T])

    nc.compile()
    _cache["nc"] = nc
    return nc


def _prep_weights(conv1_w, conv1_b, conv2_w, conv2_b, conv3_w, conv3_b,
                  fc_w, fc_b, w_ih, w_hh, b_ih, b_hh, actor_w, actor_b,
                  critic_w, critic_b):
    bf = ml_dtypes.bfloat16
    # conv1 pair-stacked block-diagonal: [ (2img x 64patch), (2img x 2rep x 32och) ]
    w1f = (conv1_w.reshape(32, 64) / 255.0).T.astype(np.float32)   # [64, 32]
    w1r2 = np.tile(w1f, (1, 2))                                    # [64, 64] 2rep
    w1p = np.zeros((128, 128), np.float32)
    w1p[0:64, 0:64] = w1r2
    w1p[64:128, 64:128] = w1r2
    b1r = np.tile(np.tile(conv1_b, 2), 2).reshape(128, 1).astype(np.float32)

    # conv2 round weights: w2l[r=(a,b)][(rp,cp,ch32), (2rep x 64och)]
    w2l = np.zeros((4, 128, 128), np.float32)
    for r in range(4):
        a, bb = divmod(r, 2)
        for rp in range(2):
            for cp in range(2):
                ki, kj = 2 * a + rp, 2 * bb + cp
                blk = conv2_w[:, :, ki, kj].T      # [ch32, och64]
                s = 2 * rp + cp
                w2l[r, 32 * s:32 * s + 32, 0:64] = blk
                w2l[r, 32 * s:32 * s + 32, 64:128] = blk
    b2r = np.tile(conv2_b, 2).reshape(128, 1).astype(np.float32)

    # conv3 round weights: w3l[r=(a,kj)][(rs,ch64), och64]; ki = 2a + rs <= 2
    w3l = np.zeros((6, 128, 64), np.float32)
    for r in range(6):
        a, kj = divmod(r, 3)
        for rs in range(2):
            ki = 2 * a + rs
            if ki <= 2:
                w3l[r, 64 * rs:64 * rs + 64, :] = conv3_w[:, :, ki, kj].T
    b3t = conv3_b.reshape(64, 1).astype(np.float32)

    fcwl = fc_w.reshape(512, 64, 49).transpose(1, 2, 0)            # [64, 49, 512]
    fcwl = fcwl.reshape(64, 49, 4, 128).transpose(2, 0, 1, 3)      # [4, 64, 49, 128]
    fcbt = fc_b.reshape(4, 128).T.astype(np.float32)
    wihl = w_ih.T.reshape(4, 128, 384).transpose(1, 0, 2).astype(np.float32)
    bihl = (b_ih + b_hh).reshape(3, 128).T.astype(np.float32)
    whhl = w_hh.T.astype(np.float32)
    aw = np.zeros((128, 8), np.float32)
    aw[:, 0:6] = actor_w.T
    aw[:, 6:7] = critic_w.T
    ab = np.zeros((8, 1), np.float32)
    ab[0:6, 0] = actor_b
    ab[6, 0] = critic_b[0]
    w2l = np.ascontiguousarray(w2l.transpose(1, 0, 2))
    w3l = np.ascontiguousarray(w3l.transpose(1, 0, 2))
    return dict(w1p=w1p.astype(bf), b1r=b1r, w2l=w2l.astype(bf), b2r=b2r,
                w3l=w3l.astype(bf), b3t=b3t, fcwl=np.ascontiguousarray(fcwl).astype(bf),
                fcbt=fcbt, wihl=wihl, bihl=bihl, whhl=whhl, awl=aw, abl=ab)


def _im2col_core(xc):
    """xc: [512, 84, 84] fp32 -> [NT, 128, NTI//2, 400] bf16, image pairs on K."""
    v = np.lib.stride_tricks.sliding_window_view(xc, (8, 8), axis=(1, 2))
    v = v[:, ::4, ::4]                              # [512, 20, 20, 8, 8]
    v = v.transpose(0, 3, 4, 1, 2)                  # [512, 8, 8, 20, 20]
    v = v.reshape(NIMG, 64, 10, 2, 10, 2)           # (yv, rp, jc, cp)
    v = v.transpose(0, 1, 3, 5, 2, 4).reshape(NIMG, 64, 400)  # (rp, cp, yv, jc)
    v = v.reshape(NT, NTI // 2, 2, 64, 400).transpose(0, 2, 3, 1, 4)
    return np.ascontiguousarray(v.reshape(NT, 128, NTI // 2, 400)).astype(ml_dtypes.bfloat16)


def kernel(x, done, gru_state, conv1_w, conv1_b, conv2_w, conv2_b, conv3_w,
           conv3_b, fc_w, fc_b, w_ih, w_hh, b_ih, b_hh, actor_w, actor_b,
           critic_w, critic_b):
    x = np.asarray(x, np.float32)
    done = np.asarray(done, np.float32)
    gru_state = np.asarray(gru_state, np.float32)
    args = [np.asarray(a, np.float32) for a in
            (conv1_w, conv1_b, conv2_w, conv2_b, conv3_w, conv3_b, fc_w, fc_b,
             w_ih, w_hh, b_ih, b_hh, actor_w, actor_b, critic_w, critic_b)]
    wmap = _prep_weights(*args)

    xr = x.reshape(T, B, 84, 84)
    dr = done.reshape(T, B)
    in_maps = []
    for c in range(NCORES):
        xc = np.ascontiguousarray(xr[:, 4 * c:4 * c + 4]).reshape(NIMG, 84, 84)
        m = dict(wmap)
        m["xcol"] = _im2col_core(xc)
        dm = (1.0 - dr[:, 4 * c:4 * c + 4]).reshape(1, T * BL).astype(np.float32)
        m["dmask"] = np.ascontiguousarray(np.broadcast_to(dm, (128, T * BL)))
        m["h0t"] = np.ascontiguousarray(gru_state[0, 4 * c:4 * c + 4].T)
        m["identw"] = np.eye(128, dtype=np.float32)
        in_maps.append(m)

    nc = build_kernel()
    import os
    kw = {}
    if os.environ.get("BASS_PROFILE"):
        kw = dict(trace=True, trace_cores=[0])
    res = run_bass_kernel_spmd(nc, in_maps, core_ids=list(range(NCORES)), **kw)
    if res.exec_time_ns is not None:
        print(f"HW exec time: {res.exec_time_ns} ns")
        if res.instructions_and_trace is not None:
            print("trace:", res.instructions_and_trace[1])

    logits = np.zeros((T * B, A_DIM), np.float32)
    value = np.zeros((T * B, 1), np.float32)
    h_final = np.zeros((1, B, H), np.float32)
    for c in range(NCORES):
        av = res.results[c]["y_av"]
        hv = res.results[c]["y_h"]
        lg = av[0:6].T.reshape(T, BL, A_DIM)
        vl = av[6].reshape(T, BL, 1)
        for e in range(BL):
            logits[np.arange(T) * B + 4 * c + e] = lg[:, e]
            value[np.arange(T) * B + 4 * c + e] = vl[:, e]
        h_final[0, 4 * c:4 * c + 4] = hv.T
    return logits, value, h_final


# revision 28
# speedup vs baseline: 1.0227x; 1.0227x over previous
"""Trainium2 Bass kernel for the Atari actor-critic agent (CNN -> GRU -> heads).

Sharding: data-parallel over the B=32 env axis -> 4 envs per core x 8 cores.
Per core: CNN on 512 frames, GRU scan over T=128 steps (4 envs), heads.
No cross-core communication.

CNN mapping (no tile_position; all matmuls full-array K=128 where possible):
 - conv1: host im2col; two images pair-stacked on K with block-diagonal
   weights -> K=128, M=[2img x 2rep x 32och].
 - conv2: act1 stored 2D-parity-split [(rowpar, colpar, ch) = 128 partitions]
   (built by 4 strip DMAs from conv1 staging); 4 rounds of K=128 accumulate.
 - conv3: act2 stored as two row-shifted strips [(shift, ch) = 128]
   (built directly by conv2's two evacuation ops); 6 rounds of K=128.
 - fc: 49 accumulating K=64 matmuls per 128-unit tile.
 - gx GEMM fp32r; GRU scan exact fp32; heads fp32.
"""
import numpy as np
import ml_dtypes

import concourse.bass as bass
from concourse import bacc
import concourse.mybir as mybir
from concourse.tile import TileContext
from concourse.bass_utils import run_bass_kernel_spmd

F32 = mybir.dt.float32
F32R = mybir.dt.float32r
BF16 = mybir.dt.bfloat16
AX = mybir.AluOpType
AF = mybir.ActivationFunctionType

T, B, BL, NCORES, H = 128, 32, 4, 8, 128
NIMG = T * BL          # 512 frames per core
NT = 16                # image tiles
NTI = NIMG // NT       # 32 frames per tile
A_DIM = 6

_cache = {}


def build_kernel():
    import os
    skip_cnn = bool(os.environ.get("SKIP_CNN"))
    skip_scan = bool(os.environ.get("SKIP_SCAN"))
    if "nc" in _cache:
        return _cache["nc"]
    nc = bacc.Bacc(None, target_bir_lowering=False, debug=False)

    # ---- DRAM inputs ----
    # conv1 im2col, image pairs stacked on K: [nt, (2img x 64patch), pair, 400]
    xcol = nc.dram_tensor("xcol", (NT, 128, NTI // 2, 400), BF16, kind="ExternalInput")
    w1p = nc.dram_tensor("w1p", (128, 128), BF16, kind="ExternalInput")
    b1r = nc.dram_tensor("b1r", (128, 1), F32, kind="ExternalInput")
    w2l = nc.dram_tensor("w2l", (128, 4, 128), BF16, kind="ExternalInput")
    b2r = nc.dram_tensor("b2r", (128, 1), F32, kind="ExternalInput")
    w3l = nc.dram_tensor("w3l", (128, 6, 64), BF16, kind="ExternalInput")
    b3t = nc.dram_tensor("b3t", (64, 1), F32, kind="ExternalInput")
    fcwl = nc.dram_tensor("fcwl", (4, 64, 49, 128), BF16, kind="ExternalInput")
    fcbt = nc.dram_tensor("fcbt", (128, 4), F32, kind="ExternalInput")
    wihl = nc.dram_tensor("wihl", (128, 4, 384), F32R, kind="ExternalInput")
    bihl = nc.dram_tensor("bihl", (128, 3), F32, kind="ExternalInput")
    whhl = nc.dram_tensor("whhl", (128, 384), F32, kind="ExternalInput")
    awl = nc.dram_tensor("awl", (128, 8), F32, kind="ExternalInput")
    abl = nc.dram_tensor("abl", (8, 1), F32, kind="ExternalInput")
    dmask = nc.dram_tensor("dmask", (128, T * BL), F32, kind="ExternalInput")
    h0t = nc.dram_tensor("h0t", (128, BL), F32, kind="ExternalInput")
    identw = nc.dram_tensor("identw", (128, 128), F32, kind="ExternalInput")

    # ---- DRAM outputs ----
    y_av = nc.dram_tensor("y_av", (8, NIMG), F32, kind="ExternalOutput")
    y_h = nc.dram_tensor("y_h", (128, BL), F32, kind="ExternalOutput")

    with TileContext(nc) as tc:
        with tc.tile_pool(name="wpool", bufs=1) as wp, \
             tc.tile_pool(name="big", bufs=1) as bigp:
            w1_s = wp.tile([128, 128], BF16); nc.sync.dma_start(w1_s[:], w1p[:])
            b1_s = wp.tile([128, 1], F32); nc.sync.dma_start(b1_s[:], b1r[:])
            w2_s = wp.tile([128, 4, 128], BF16); nc.sync.dma_start(w2_s[:], w2l[:])
            b2_s = wp.tile([128, 1], F32); nc.sync.dma_start(b2_s[:], b2r[:])
            w3_s = wp.tile([128, 6, 64], BF16); nc.sync.dma_start(w3_s[:], w3l[:])
            b3_s = wp.tile([64, 1], F32); nc.sync.dma_start(b3_s[:], b3t[:])
            fcb_s = wp.tile([128, 4], F32); nc.sync.dma_start(fcb_s[:], fcbt[:])
            fcw_s = wp.tile([64, 4, 49, 128], BF16)
            nc.sync.dma_start(fcw_s[:], fcwl.rearrange("m p a b -> p m a b"))
            wih_s = wp.tile([128, 4, 384], F32R); nc.sync.dma_start(wih_s[:], wihl[:])
            bih_s = wp.tile([128, 3], F32); nc.sync.dma_start(bih_s[:], bihl[:])
            whh_s = wp.tile([128, 384], F32); nc.sync.dma_start(whh_s[:], whhl[:])
            aw_s = wp.tile([128, 8], F32); nc.sync.dma_start(aw_s[:], awl[:])
            ab_s = wp.tile([8, 1], F32); nc.sync.dma_start(ab_s[:], abl[:])
            dm_s = bigp.tile([128, T * BL], F32); nc.sync.dma_start(dm_s[:], dmask[:])
            h0_s = wp.tile([128, BL], F32); nc.sync.dma_start(h0_s[:], h0t[:])
            id_s = wp.tile([128, 128], F32); nc.sync.dma_start(id_s[:], identw[:])

            act3_s = bigp.tile([64, 2, 64, 49], BF16)  # rolling 2-chunk
            feat_s = bigp.tile([128, 4, NIMG], F32R)
            gx_s = bigp.tile([128, T, 3, BL], F32)
            hs_s = bigp.tile([128, T * BL], F32)
            ns_s = bigp.tile([128, T * BL], F32)
            hm_s = bigp.tile([128, T * BL], F32)
            rz_s = bigp.tile([128, T, 8], F32)

            # ---------------- CNN emission as a generator ----------------
            # yields between op groups so scan steps can interleave
            CH = 2              # nt tiles per fc chunk (64 images, 16 steps)
            NCHUNK = NT // CH   # 8 chunks
            gen_done = [False] * NCHUNK

            def emit_cnn_chunk(ck, pools):
                xinp, st1p, as1p, as2p, ps1p, ps2p, ps3p, psfc, psgx = pools
                for nt in range(CH * ck, CH * (ck + 1)):
                    xin = xinp.tile([128, NTI // 2, 400], BF16)
                    nc.sync.dma_start(xin[:], xcol[nt])
                    st1 = st1p.tile([128, NTI // 2, 400], BF16)
                    for pr in range(NTI // 2):
                        ps = ps1p.tile([128, 400], F32)
                        nc.tensor.matmul(ps[:], w1_s[:], xin[:, pr, :],
                                         start=True, stop=True)
                        if pr % 4 != 3:
                            nc.scalar.activation(st1[:, pr, :], ps[:], AF.Relu,
                                                 bias=b1_s[:, 0:1])
                        else:
                            nc.vector.tensor_scalar(st1[:, pr, :], ps[:],
                                                    b1_s[:, 0:1], 0.0, AX.add, AX.max)
                        if pr % 4 == 3:
                            yield
                    as1 = as1p.tile([128, NTI, 10, 10], BF16)
                    st1v = st1.rearrange("p a (u v y x) -> p a u v (y x)", u=2, v=2, y=10, x=10)
                    a1v = as1.rearrange("p (a h) y j -> p a h (y j)", h=2)
                    for rp in range(2):
                        for cp in range(2):
                            sp = 2 * rp + cp
                            for half in range(2):
                                p0 = 64 * half + 32 * (sp % 2)
                                nc.sync.dma_start(a1v[32 * sp:32 * sp + 32, :, half, :],
                                                  st1v[p0:p0 + 32, :, rp, cp, :])
                    yield
                    as2 = as2p.tile([128, NTI, 9, 9], BF16)
                    nc.vector.memset(as2[64:128, :, 8, :], 0.0)
                    for cc in range((NTI + 5) // 6):
                        i0 = 6 * cc
                        isz = min(6, NTI - i0)
                        ncols = isz * 81
                        ps2 = ps2p.tile([128, 512], F32)
                        for r in range(4):
                            a, bb = divmod(r, 2)
                            rhs = as1[:, i0:i0 + isz, a:a + 9, bb:bb + 9]
                            nc.tensor.matmul(ps2[:, :ncols], w2_s[:, r, :], rhs,
                                             start=(r == 0), stop=(r == 3))
                        p2v = ps2[:, :ncols].rearrange("p (a y x) -> p a y x", a=isz, y=9)
                        nc.scalar.activation(as2[0:64, i0:i0 + isz], p2v[0:64], AF.Relu,
                                             bias=b2_s[0:64, 0:1])
                        if cc % 3 == 2:
                            nc.vector.tensor_scalar(as2[64:128, i0:i0 + isz, 0:8, :],
                                                    p2v[64:128, :, 1:9, :],
                                                    b2_s[64:128, 0:1], 0.0, AX.add, AX.max)
                        else:
                            nc.scalar.activation(as2[64:128, i0:i0 + isz, 0:8, :],
                                                 p2v[64:128, :, 1:9, :], AF.Relu,
                                                 bias=b2_s[64:128, 0:1])
                        yield
                    for cc in range((NTI + 9) // 10):
                        i0 = 10 * cc
                        isz = min(10, NTI - i0)
                        ncols = isz * 49
                        ps3 = ps3p.tile([128, 512], F32)
                        for r in range(6):
                            a, kj = divmod(r, 3)
                            rhs = as2[:, i0:i0 + isz, 2 * a:2 * a + 7, kj:kj + 7]
                            nc.tensor.matmul(ps3[0:64, :ncols], w3_s[:, r, :], rhs,
                                             start=(r == 0), stop=(r == 5))
                        ck_slot = (nt // CH) % 2
                        io = (nt % CH) * NTI + i0
                        dst = act3_s[:, ck_slot, io:io + isz, :]
                        dst = dst.rearrange("p a b -> p (a b)")
                        if cc % 3 != 2:
                            nc.scalar.activation(dst, ps3[0:64, :ncols], AF.Relu,
                                                 bias=b3_s[:, 0:1])
                        else:
                            nc.vector.tensor_scalar(dst, ps3[0:64, :ncols],
                                                    b3_s[:, 0:1], 0.0, AX.add, AX.max)
                        yield
                # fc + gx for this chunk (images [NI0, NI1))
                NI0 = CH * NTI * ck
                NW = CH * NTI
                for m in range(4):
                    ps = psfc.tile([128, NW], F32)
                    for pos in range(49):
                        nc.tensor.matmul(ps[:], fcw_s[:, m, pos, :],
                                         act3_s[:, ck % 2, :, pos],
                                         start=(pos == 0), stop=(pos == 48))
                    nc.scalar.activation(feat_s[:, m, NI0:NI0 + NW], ps[:], AF.Relu,
                                         bias=fcb_s[:, m:m + 1])
                    yield
                for g in range(3):
                    ps = psgx.tile([128, NW], F32)
                    for kt in range(4):
                        nc.tensor.matmul(ps[:], wih_s[:, kt, 128 * g:128 * (g + 1)],
                                         feat_s[:, kt, NI0:NI0 + NW],
                                         start=(kt == 0), stop=(kt == 3))
                    nc.vector.tensor_scalar(
                        gx_s[:, 16 * CH * ck // CH * 0 + NI0 // BL:NI0 // BL + NW // BL, g, :],
                        ps[:].rearrange("p (t e) -> p t e", e=BL),
                        bih_s[:, g:g + 1], 0.0, AX.add, AX.bypass)
                    yield
                gen_done[ck] = True
                while True:
                    yield

            # ---------------- GRU scan, CNN interleaved ----------------
            with tc.tile_pool(name="xin", bufs=2) as xinp, \
                 tc.tile_pool(name="st1", bufs=2) as st1p, \
                 tc.tile_pool(name="as1", bufs=2) as as1p, \
                 tc.tile_pool(name="as2", bufs=2) as as2p, \
                 tc.tile_pool(name="ps1", bufs=2, space="PSUM") as ps1p, \
                 tc.tile_pool(name="ps2", bufs=1, space="PSUM") as ps2p, \
                 tc.tile_pool(name="ps3", bufs=1, space="PSUM") as ps3p, \
                 tc.tile_pool(name="psfc", bufs=1, space="PSUM") as psfc, \
                 tc.tile_pool(name="psgx", bufs=1, space="PSUM") as psgx, \
                 tc.tile_pool(name="pssc", bufs=2, space="PSUM") as pssc, \
                 tc.tile_pool(name="scw", bufs=6) as scw, \
                 tc.tile_pool(name="bp", bufs=1) as bpp:
                pools = (xinp, st1p, as1p, as2p, ps1p, ps2p, ps3p, psfc, psgx)
                gens = [emit_cnn_chunk(ck, pools) for ck in range(NCHUNK)]
                if skip_cnn:
                    nc.vector.memset(gx_s[:, :, :, :], 0.0)
                else:
                    for _ in range(200):
                        next(gens[0])
                SPC = T // NCHUNK  # scan steps per chunk
                # hm(0) = h0 * dmask(0)
                nc.vector.tensor_tensor(hm_s[:, 0:BL], h0_s[:], dm_s[:, 0:BL], AX.mult)
                for t in range(T if not skip_scan else 1):
                    hm = hm_s[:, BL * t:BL * (t + 1)]
                    ps = pssc.tile([128, 12], F32)
                    for g in range(3):
                        nc.tensor.matmul(ps[:, 4 * g:4 * (g + 1)],
                                         whh_s[:, 128 * g:128 * (g + 1)], hm,
                                         start=True, stop=True)
                    srz = scw.tile([128, 8], F32, tag="srz")
                    nc.vector.tensor_tensor(
                        srz[:], ps[:, 0:8],
                        gx_s[:, t, 0:2, :].rearrange("p a b -> p (a b)"), AX.add)
                    rz = rz_s[:, t, :]
                    nc.scalar.activation(rz, srz[:], AF.Sigmoid)
                    t1 = scw.tile([128, BL], F32, tag="t1")
                    nc.vector.tensor_tensor(t1[:], ps[:, 8:12], rz_s[:, t, 0:4], AX.mult)
                    t2 = scw.tile([128, BL], F32, tag="t2")
                    nc.vector.tensor_tensor(t2[:], t1[:], gx_s[:, t, 2, :], AX.add)
                    nsb = ns_s[:, BL * t:BL * (t + 1)]
                    nc.scalar.activation(nsb, t2[:], AF.Tanh)
                    if t + 1 < T:
                        # hm(t+1) = dm1*(n + z*(hm - n)) = n*q + p,
                        # q = (1-z)*dm1, p = z*dm1*hm   (q, p off critical path)
                        dm1 = dm_s[:, BL * (t + 1):BL * (t + 2)]
                        za = scw.tile([128, BL], F32, tag="za")
                        nc.vector.tensor_tensor(za[:], rz_s[:, t, 4:8], dm1, AX.mult)
                        qd = scw.tile([128, BL], F32, tag="qd")
                        nc.vector.tensor_tensor(qd[:], dm1, za[:], AX.subtract)
                        pd = scw.tile([128, BL], F32, tag="pd")
                        nc.vector.tensor_tensor(pd[:], za[:], hm, AX.mult)
                        u = scw.tile([128, BL], F32, tag="u")
                        nc.vector.tensor_tensor(u[:], nsb, qd[:], AX.mult)
                        nc.vector.tensor_tensor(hm_s[:, BL * (t + 1):BL * (t + 2)],
                                                u[:], pd[:], AX.add)
                    if not skip_cnn:
                        ck_next = t // SPC + 1
                        if ck_next < NCHUNK:
                            g = gens[ck_next]
                            for _ in range(3):
                                try:
                                    next(g)
                                except StopIteration:
                                    break
                assert all(gen_done), f"CNN chunk emission incomplete: {gen_done}"
                # batched hs = ns + z*(hm - ns)
                for seg in range(2):
                    sl = slice(seg * T * BL // 2, (seg + 1) * T * BL // 2)
                    tsl = slice(seg * T // 2, (seg + 1) * T // 2)
                    bv = bpp.tile([128, T * BL // 2], F32, tag="bv")
                    nc.vector.tensor_tensor(bv[:], hm_s[:, sl], ns_s[:, sl], AX.subtract)
                    bw = bpp.tile([128, T * BL // 2], F32, tag="bw")
                    nc.vector.tensor_tensor(
                        bw[:], bv[:].rearrange("p (t e) -> p t e", e=BL),
                        rz_s[:, tsl, 4:8], AX.mult)
                    nc.vector.tensor_tensor(hs_s[:, sl], bw[:], ns_s[:, sl], AX.add)

            # ---------------- heads ----------------
            with tc.tile_pool(name="psh", bufs=1, space="PSUM") as pshp, \
                 tc.tile_pool(name="ho", bufs=1) as hop:
                psh = pshp.tile([8, NIMG], F32)
                nc.tensor.matmul(psh[:], aw_s[:], hs_s[:], start=True, stop=True)
                osb = hop.tile([8, NIMG], F32)
                nc.vector.tensor_scalar(osb[:], psh[:], ab_s[:, 0:1], 0.0,
                                        AX.add, AX.bypass)
                nc.sync.dma_start(y_av[:], osb[:])
                nc.sync.dma_start(y_h[:], hs_s[:, BL * (T - 1):BL * T])

    nc.compile()
    _cache["nc"] = nc
    return nc


def _prep_weights(conv1_w, conv1_b, conv2_w, conv2_b, conv3_w, conv3_b,
                  fc_w, fc_b, w_ih, w_hh, b_ih, b_hh, actor_w, actor_b,
                  critic_w, critic_b):
    bf = ml_dtypes.bfloat16
    # conv1 pair-stacked block-diagonal: [ (2img x 64patch), (2img x 2rep x 32och) ]
    w1f = (conv1_w.reshape(32, 64) / 255.0).T.astype(np.float32)   # [64, 32]
    w1r2 = np.tile(w1f, (1, 2))                                    # [64, 64] 2rep
    w1p = np.zeros((128, 128), np.float32)
    w1p[0:64, 0:64] = w1r2
    w1p[64:128, 64:128] = w1r2
    b1r = np.tile(np.tile(conv1_b, 2), 2).reshape(128, 1).astype(np.float32)

    # conv2 round weights: w2l[r=(a,b)][(rp,cp,ch32), (2rep x 64och)]
    w2l = np.zeros((4, 128, 128), np.float32)
    for r in range(4):
        a, bb = divmod(r, 2)
        for rp in range(2):
            for cp in range(2):
                ki, kj = 2 * a + rp, 2 * bb + cp
                blk = conv2_w[:, :, ki, kj].T      # [ch32, och64]
                s = 2 * rp + cp
                w2l[r, 32 * s:32 * s + 32, 0:64] = blk
                w2l[r, 32 * s:32 * s + 32, 64:128] = blk
    b2r = np.tile(conv2_b, 2).reshape(128, 1).astype(np.float32)

    # conv3 round weights: w3l[r=(a,kj)][(rs,ch64), och64]; ki = 2a + rs <= 2
    w3l = np.zeros((6, 128, 64), np.float32)
    for r in range(6):
        a, kj = divmod(r, 3)
        for rs in range(2):
            ki = 2 * a + rs
            if ki <= 2:
                w3l[r, 64 * rs:64 * rs + 64, :] = conv3_w[:, :, ki, kj].T
    b3t = conv3_b.reshape(64, 1).astype(np.float32)

    fcwl = fc_w.reshape(512, 64, 49).transpose(1, 2, 0)            # [64, 49, 512]
    fcwl = fcwl.reshape(64, 49, 4, 128).transpose(2, 0, 1, 3)      # [4, 64, 49, 128]
    fcbt = fc_b.reshape(4, 128).T.astype(np.float32)
    wihl = w_ih.T.reshape(4, 128, 384).transpose(1, 0, 2).astype(np.float32)
    bihl = (b_ih + b_hh).reshape(3, 128).T.astype(np.float32)
    whhl = w_hh.T.astype(np.float32)
    aw = np.zeros((128, 8), np.float32)
    aw[:, 0:6] = actor_w.T
    aw[:, 6:7] = critic_w.T
    ab = np.zeros((8, 1), np.float32)
    ab[0:6, 0] = actor_b
    ab[6, 0] = critic_b[0]
    w2l = np.ascontiguousarray(w2l.transpose(1, 0, 2))
    w3l = np.ascontiguousarray(w3l.transpose(1, 0, 2))
    return dict(w1p=w1p.astype(bf), b1r=b1r, w2l=w2l.astype(bf), b2r=b2r,
                w3l=w3l.astype(bf), b3t=b3t, fcwl=np.ascontiguousarray(fcwl).astype(bf),
                fcbt=fcbt, wihl=wihl, bihl=bihl, whhl=whhl, awl=aw, abl=ab)


def _im2col_core(xc):
    """xc: [512, 84, 84] fp32 -> [NT, 128, NTI//2, 400] bf16, image pairs on K."""
    v = np.lib.stride_tricks.sliding_window_view(xc, (8, 8), axis=(1, 2))
    v = v[:, ::4, ::4]                              # [512, 20, 20, 8, 8]
    v = v.transpose(0, 3, 4, 1, 2)                  # [512, 8, 8, 20, 20]
    v = v.reshape(NIMG, 64, 10, 2, 10, 2)           # (yv, rp, jc, cp)
    v = v.transpose(0, 1, 3, 5, 2, 4).reshape(NIMG, 64, 400)  # (rp, cp, yv, jc)
    v = v.reshape(NT, NTI // 2, 2, 64, 400).transpose(0, 2, 3, 1, 4)
    return np.ascontiguousarray(v.reshape(NT, 128, NTI // 2, 400)).astype(ml_dtypes.bfloat16)


def kernel(x, done, gru_state, conv1_w, conv1_b, conv2_w, conv2_b, conv3_w,
           conv3_b, fc_w, fc_b, w_ih, w_hh, b_ih, b_hh, actor_w, actor_b,
           critic_w, critic_b):
    x = np.asarray(x, np.float32)
    done = np.asarray(done, np.float32)
    gru_state = np.asarray(gru_state, np.float32)
    args = [np.asarray(a, np.float32) for a in
            (conv1_w, conv1_b, conv2_w, conv2_b, conv3_w, conv3_b, fc_w, fc_b,
             w_ih, w_hh, b_ih, b_hh, actor_w, actor_b, critic_w, critic_b)]
    wmap = _prep_weights(*args)

    xr = x.reshape(T, B, 84, 84)
    dr = done.reshape(T, B)
    in_maps = []
    for c in range(NCORES):
        xc = np.ascontiguousarray(xr[:, 4 * c:4 * c + 4]).reshape(NIMG, 84, 84)
        m = dict(wmap)
        m["xcol"] = _im2col_core(xc)
        dm = (1.0 - dr[:, 4 * c:4 * c + 4]).reshape(1, T * BL).astype(np.float32)
        m["dmask"] = np.ascontiguousarray(np.broadcast_to(dm, (128, T * BL)))
        m["h0t"] = np.ascontiguousarray(gru_state[0, 4 * c:4 * c + 4].T)
        m["identw"] = np.eye(128, dtype=np.float32)
        in_maps.append(m)

    nc = build_kernel()
    import os
    kw = {}
    if os.environ.get("BASS_PROFILE"):
        kw = dict(trace=True, trace_cores=[0])
    res = run_bass_kernel_spmd(nc, in_maps, core_ids=list(range(NCORES)), **kw)
    if res.exec_time_ns is not None:
        print(f"HW exec time: {res.exec_time_ns} ns")
        if res.instructions_and_trace is not None:
            print("trace:", res.instructions_and_trace[1])

    logits = np.zeros((T * B, A_DIM), np.float32)
    value = np.zeros((T * B, 1), np.float32)
    h_final = np.zeros((1, B, H), np.float32)
    for c in range(NCORES):
        av = res.results[c]["y_av"]
        hv = res.results[c]["y_h"]
        lg = av[0:6].T.reshape(T, BL, A_DIM)
        vl = av[6].reshape(T, BL, 1)
        for e in range(BL):
            logits[np.arange(T) * B + 4 * c + e] = lg[:, e]
            value[np.arange(T) * B + 4 * c + e] = vl[:, e]
        h_final[0, 4 * c:4 * c + 4] = hv.T
    return logits, value, h_final
